# revision 5
# baseline (speedup 1.0000x reference)
"""MeshConvPoint Trainium2 kernel (8-core SPMD), SBUF-resident gather.

Math: per vertex v with gathered features f0..f3 (4 random indices/vertex):
  out = einsum(G, W) + b over G = [f0, p1, e3, e2, p2, 2(mx-mn), p3]
where p_k = sum_j f_j^k over f1..f3.  All symmetric functions reduce to
8 features [f0, p1, p2, p3, p1^2, p1^3, p1*p2, mx-mn]; the linear
recombination + scale factors are folded into the weights host-side.

Sharding: 8 cores = 4 batches x 2 vertex halves (data parallel, no
collectives).

Gather: the two compacted feature tables (unique rows referenced by index
slots {0,1} and {2,3}; each row 128 fp16 = [x(64ch) | x^2(64ch)]) live
RESIDENT IN SBUF.  dma_gather(transpose=True) with SBUF source fetches each
row as a 128-partition column, so gathered features arrive channel-major:
no HBM random reads, no DRAM staging bounce, no PE transposes.  Row u sits
at partition u%128, free offset (u//128)*256B (sbuf_tokens_per_rank=128).
"""

import sys

sys.path.insert(0, "/opt/trn_rl_repo")

import numpy as np

import concourse.bass as bass
import concourse.tile as tile
from concourse import bacc, mybir
from concourse.bass_utils import run_bass_kernel_spmd

B, C, V, CO, K = 4, 64, 50000, 128, 7
VPC = 25088          # padded vertices per core (2 halves of 50000 -> 196*128)
NV = 1792            # vertices per gather block (multiple of 128)
NBLK = VPC // NV     # 14
SG = 448             # matmul supertile columns (448*4B = 1792B <= 2KB PSUM bank)
NSG = NV // SG       # 4
TBLR = 32768         # table capacity in rows (int16 index space)
RANKS = TBLR // 128  # 256 rank stripes of 256B per partition
F16 = mybir.dt.float16
F32 = mybir.dt.float32
I16 = mybir.dt.int16

_cache = {}


def build_program(loop_iters=1):
    key = loop_iters
    if key in _cache:
        return _cache[key]
    nc = bacc.Bacc("TRN2", target_bir_lowering=False, debug=False, num_devices=8)
    tblA_d = nc.dram_tensor("tblA", [128, RANKS * 128], F16, kind="ExternalInput").ap()
    tblB_d = nc.dram_tensor("tblB", [128, RANKS * 128], F16, kind="ExternalInput").ap()
    idx_d = nc.dram_tensor("idx", [128, 4 * VPC // 16], I16, kind="ExternalInput").ap()
    wch = nc.dram_tensor("wch", [4, 128, 128], F16, kind="ExternalInput").ap()
    bias = nc.dram_tensor("bias", [128, 1], F32, kind="ExternalInput").ap()
    out = nc.dram_tensor("out", [128, VPC], F16, kind="ExternalOutput").ap()

    with tile.TileContext(nc) as tc:
        import contextlib

        with contextlib.ExitStack() as ctx:
            cst = ctx.enter_context(tc.tile_pool(name="cst", bufs=1))
            gpl = ctx.enter_context(tc.tile_pool(name="g", bufs=2))
            ftp = ctx.enter_context(tc.tile_pool(name="ft", bufs=1))
            pop = ctx.enter_context(tc.tile_pool(name="po", bufs=2, space="PSUM"))
            otp = ctx.enter_context(tc.tile_pool(name="ot", bufs=3))

            tblA_sb = cst.tile([128, RANKS * 128], F16)
            nc.sync.dma_start(out=tblA_sb[:], in_=tblA_d[:])
            tblB_sb = cst.tile([128, RANKS * 128], F16)
            nc.sync.dma_start(out=tblB_sb[:], in_=tblB_d[:])
            idx_sb = cst.tile([128, 4 * VPC // 16], I16)
            nc.sync.dma_start(out=idx_sb[:], in_=idx_d[:])
            w_sb = []
            for j in range(4):
                wt = cst.tile([128, 128], F16, tag=f"w{j}", name=f"w{j}")
                w_sb.append(wt)
            for j in range(4):
                nc.sync.dma_start(out=w_sb[j][:], in_=wch[j])
            bias_sb = cst.tile([128, 1], F32)
            nc.sync.dma_start(out=bias_sb[:], in_=bias[:])

            tt = nc.vector.tensor_tensor
            op = mybir.AluOpType

            def top(t):
                return t[0:64, :]

            def bot(t):
                return t[64:128, :]

            def block(c):
                gs = []
                for j in range(4):
                    g = gpl.tile([128, NV], F16, tag=f"g{j}", name=f"g{j}_{c}")
                    tbl = tblA_sb if j < 2 else tblB_sb
                    col0 = j * (VPC // 16) + c * (NV // 16)
                    nc.gpsimd.dma_gather(
                        out_ap=g[:].rearrange("p (k v) -> p k v", k=1),
                        in_ap=tbl[:],
                        idxs_ap=idx_sb[:, col0 : col0 + NV // 16],
                        num_idxs=NV,
                        num_idxs_reg=NV,
                        elem_size=128,
                        transpose=True,
                        single_packet=False,
                        sbuf_tokens_per_rank=128,
                        sbuf_free_dim_per_rank=256,
                    )
                    gs.append(g)
                g0, g1, g2, g3 = gs
                # NOTE: neuronxcc requires both SBUF inputs of a tensor_tensor
                # to share the same base partition; outputs may differ.  All
                # ops below read base-0 (or full-tile) inputs only.
                T1 = ftp.tile([128, NV], F16, tag="T1")
                T2 = ftp.tile([128, NV], F16, tag="T2")
                T3 = ftp.tile([128, NV], F16, tag="T3")
                T4 = ftp.tile([128, NV], F16, tag="T4")
                MM = ftp.tile([128, NV], F16, tag="MM")
                PP = ftp.tile([128, NV], F16, tag="PP")
                # T1 = [p1 ; p3] (cubes ride in bottom halves of g)
                tt(out=T1[:], in0=g1[:], in1=g2[:], op=op.add)
                tt(out=T1[:], in0=T1[:], in1=g3[:], op=op.add)
                # p2 = sum x_j^2 -> top(T2)
                tt(out=top(T2), in0=top(g1), in1=top(g1), op=op.mult)
                tt(out=top(PP), in0=top(g2), in1=top(g2), op=op.mult)
                tt(out=top(T2), in0=top(T2), in1=top(PP), op=op.add)
                tt(out=top(PP), in0=top(g3), in1=top(g3), op=op.mult)
                tt(out=top(T2), in0=top(T2), in1=top(PP), op=op.add)
                # mxd = max - min over x -> bot(T2)
                tt(out=top(MM), in0=top(g1), in1=top(g2), op=op.max)
                tt(out=top(MM), in0=top(MM), in1=top(g3), op=op.max)
                tt(out=top(PP), in0=top(g1), in1=top(g2), op=op.min)
                tt(out=top(PP), in0=top(PP), in1=top(g3), op=op.min)
                tt(out=bot(T2), in0=top(MM), in1=top(PP), op=op.subtract)
                # T3 = [p1^2 ; p1^3]
                tt(out=top(T3), in0=top(T1), in1=top(T1), op=op.mult)
                tt(out=bot(T3), in0=top(T3), in1=top(T1), op=op.mult)
                # T4 = [f0 ; p1*p2]
                tt(out=bot(T4), in0=top(T1), in1=top(T2), op=op.mult)
                nc.scalar.activation(
                    out=top(T4), in_=top(g0), func=mybir.ActivationFunctionType.Copy
                )
                for st in range(NSG):
                    sl = slice(st * SG, (st + 1) * SG)
                    psO = pop.tile([128, SG], F32, tag="psO")
                    nc.tensor.matmul(out=psO[:], lhsT=w_sb[0][:], rhs=T1[:, sl], start=True, stop=False)
                    nc.tensor.matmul(out=psO[:], lhsT=w_sb[1][:], rhs=T2[:, sl], start=False, stop=False)
                    nc.tensor.matmul(out=psO[:], lhsT=w_sb[2][:], rhs=T3[:, sl], start=False, stop=False)
                    nc.tensor.matmul(out=psO[:], lhsT=w_sb[3][:], rhs=T4[:, sl], start=False, stop=True)
                    ot = otp.tile([128, SG], F16, tag="ot")
                    nc.scalar.activation(
                        out=ot[:],
                        in_=psO[:],
                        func=mybir.ActivationFunctionType.Identity,
                        bias=bias_sb[:],
                    )
                    v0 = c * NV + st * SG
                    nc.sync.dma_start(out=out[:, v0 : v0 + SG], in_=ot[:])

            def body():
                for c in range(NBLK):
                    block(c)

            if loop_iters == 1:
                body()
            else:
                with tc.For_i(0, loop_iters, 1) as _:
                    body()

    nc.compile()
    _cache[key] = nc
    return nc


def prep_inputs(x, Gi, W, b):
    """Host-side sharding/packing. Returns list of 8 per-core input maps."""
    x = np.asarray(x)
    Gi = np.asarray(Gi)
    W = np.asarray(W, dtype=np.float32)
    b = np.asarray(b, dtype=np.float32)
    xs = x[..., 0].astype(np.float32)  # [B, C, V]

    # weight recombination (scale factors folded in)
    W0, W1, W2, W3, W4, W5, W6 = [W[:, :, k] for k in range(7)]  # each [CO, C]
    feats = {
        "p1": W1, "p2": W4 - W3 / 2, "p3": W6 + W2 / 3, "mxd": 2 * W5,
        "sq": W3 / 2, "cu": W2 / 6, "pp": -W2 / 2, "f0": W0,
    }
    pairs = [("p1", "p3"), ("p2", "mxd"), ("sq", "cu"), ("f0", "pp")]
    wch = np.zeros((4, 128, 128), dtype=np.float16)
    for j, (lo, hi) in enumerate(pairs):
        wch[j, 0:64, :] = feats[lo].T.astype(np.float16)
        wch[j, 64:128, :] = feats[hi].T.astype(np.float16)
    bias = b.reshape(128, 1).astype(np.float32)

    rows_b = []
    for bb in range(B):
        x16 = np.ascontiguousarray(xs[bb].T).astype(np.float16)     # [V, C]
        cu16 = (x16.astype(np.float32) ** 3).astype(np.float16)     # [V, C]
        rows_b.append(np.concatenate([x16, cu16], axis=1))          # [V, 128] f16

    def pack_tbl(rows):
        # rows [n<=TBLR, 128] -> SBUF layout [128, RANKS*128]:
        # row u -> partition u%128, fp16 offset (u//128)*128
        tb = np.zeros((TBLR, 128), dtype=np.float16)
        tb[: len(rows)] = rows
        return np.ascontiguousarray(
            tb.reshape(RANKS, 128, 128).transpose(1, 0, 2).reshape(128, RANKS * 128)
        )

    def pack_idx(stream):
        # [VPC] int -> wrapped [128, VPC//16]: block-local idx i at
        # partition i%16, col (block base + i//16); replicated x8.
        blocks = []
        for c in range(NBLK):
            blk = stream[c * NV : (c + 1) * NV]
            blocks.append(blk.reshape(NV // 16, 16).T)    # [16, NV//16]
        cols = np.concatenate(blocks, axis=1)             # [16, VPC//16]
        return np.ascontiguousarray(np.tile(cols, (8, 1)).astype(np.int16))

    maps = []
    for core in range(8):
        bb, h = divmod(core, 2)
        v0 = h * VPC
        nreal = min(VPC, V - v0)
        gi = np.zeros((VPC, 4), dtype=np.int64)
        gi[:nreal] = Gi[bb, v0 : v0 + nreal, :]
        m = {"wch": wch, "bias": bias}
        streams = [None] * 4
        for nm, sl in (("A", (0, 1)), ("B", (2, 3))):
            u, inv = np.unique(gi[:, sl[0] : sl[1] + 1], return_inverse=True)
            assert len(u) <= TBLR, len(u)
            inv = inv.reshape(VPC, 2)
            m["tbl" + nm] = pack_tbl(rows_b[bb][u])
            streams[sl[0]] = inv[:, 0]
            streams[sl[1]] = inv[:, 1]
        m["idx"] = np.concatenate([pack_idx(s) for s in streams], axis=1)
        maps.append(m)
    return maps


def assemble(results):
    out = np.zeros((B, CO, V, 1), dtype=np.float32)
    for core in range(8):
        bb, h = divmod(core, 2)
        v0 = h * VPC
        nreal = min(VPC, V - v0)
        o = results[core]["out"].astype(np.float32)  # [128, VPC]
        out[bb, :, v0 : v0 + nreal, 0] = o[:, :nreal]
    return out


def kernel(**inputs):
    nc = build_program(1)
    maps = prep_inputs(inputs["x"], inputs["Gi"], inputs["W"], inputs["b"])
    res = run_bass_kernel_spmd(nc, maps, list(range(8)))
    return assemble(res.results)
